# revision 40
# baseline (speedup 1.0000x reference)
"""DNDF tree (soft decision tree / dense MoE) kernel for Trainium2.

Full computation (reference):
    dprob  = sigmoid(x @ Wd.T + bd)                 [B, 63]
    routing[b, l] = prod_d (pos ? dprob[idx] : 1 - dprob[idx])   [B, 64]
    leaves = softmax(einsum('bi,loi->blo', x, Wl) + bl, axis=-1) [B, 64, O]
    out    = einsum('bl,blo->bo', routing, leaves)  [B, O]

Sharding: expert-parallel over the 64 leaves: core c owns leaves
8c..8c+7, computes partial = sum_{l in core} routing[:, l] * leaves[:, l, :]
over the FULL batch; the host sums the 8 per-core partials.

Routing trick (no gathers): with z = x@Wd.T + bd,
    log p      = -softplus(-z),   log(1-p) = -softplus(z)
    log routing[b,l] = -( [softplus(-z); softplus(z)] @ [A; B] )[b, l]
where A[n,l]=1 iff leaf l visits node n on the sigmoid branch and B for
the (1-sigmoid) branch - ONE 126-K matmul per batch tile against the
host-stacked [A; B] ([softplus(-z) is computed in place in rows 0:63 of
the stacked tile; softplus(z) is DMA-copied into rows 63:126 - compute
engines cannot cross partitions but DMA can).  softplus(t) = Ln(Exp(t)+1)
with the one ACT table set that has both exp and ln; all four Exp chunks
run before the four Ln chunks so the table set is not swapped (~1.3us)
per chunk.  routing-r runs AFTER expert 0 (only experts 1+ consume r),
off the DMA-saturated startup window.

Routing z runs K[0:256] on the fp8 path (one DoubleRow matmul against
the x8 tile's kc0 slice; wd8 zero-padded to 64 nodes - the dual-fp8
LDWEIGHTS ISA check rejects an odd stationary column count) and
K[256:1024] in f16.  This trims the startup-critical f16 x tensor from
4MB to 3MB (the window is DMA-byte-bound) and costs 1.64e-2 -> 1.77e-2
rel err (sim 1.743e-2; full half-K fp8 routing measured 2.07e-2, over
the gate).  The z PSUM is then at 1024x scale: the softplus Exp folds
in 1/1024 and the softplus(-z) subtraction becomes a DVE stt with a
-1/1024 per-partition scalar.

Adaptive per-(core, batch-row) precision ("ada8"): the output error a
core's fp8 noise causes at a row scales with that core's routing weight
there, and the max-err statistic is dominated by a handful of rows where
one leaf's routing weight is large.  So the host ranks rows per core by
max leaf routing weight (a cheap [B,63] GEMM + softplus in numpy),
permutes the batch per core so the top-256 rows land in batch tiles 0-1,
and the device runs:
  - tiles 0-1 ("hyb512"):  K 0..511 fp8e4 DoubleRow + K 512..1023 f16
  - tiles 2-15 ("fp8"):    all 1024 K as 4 fp8 DoubleRow matmuls
That is 1088 PE matmuls per core instead of 1536 (-29%).  Measured HW
rel err 1.638e-2 (numpy sim of the full quantization chain predicted
1.640e-2; the sim tracks HW within ~2%).  Uniform K8=768 (2.17e-2),
mixing per *expert* (2.2e-2+), fp8 routing (2.07e-2 even half-K), and
only 1 protected tile (2.2e-2) all breach the 2e-2 gate - per-row
targeting with 2 protected tiles is what keeps the max error pinned.
f16 accumulators add 4e-4 (1.602->1.640 sim) and halve DVE/SBUF cost.

Scales: x8 = e4m3(16x), w8 = e4m3(64w), w16 = f16(1024w) so all paths
accumulate logits at 1024x; the softmax exp folds in scale=1/1024.

PE stream discipline (the big wins, measured):
  - Stationary reuse: kc-outer/bt-inner matmul order within a 2-tile
    group streams each 256-row DR weight load across two 512-col passes;
    a fresh-per-matmul DR load costs +40-60ns over the 213ns stream
    (272ns avg -> 216ns avg, -56us).  Needs 4 live [128,1024] PSUM logit
    tiles = all 8 banks, so routing z/r tiles are carved from the same
    single PSUM pool (warm-up included - a stale handle would alias).
  - ONE fused ScalarE activation per block (exp over both 512-col halves
    of the 2-bank PSUM tile, accum_out for the softmax denominator):
    (1024+352)/1.2 ~= 1.1us, vs 2x720 + extra READ_ACCUM unfused.
  - Expert 0 parks unscaled exp in the f16 accumulator via its eviction
    activation (pure ScalarE); expert 1 applies the deferred scale, so
    expert 0 can run before routing-r exists.
  - Writeback: one full-width [128,1024] f16 DMA per batch tile (2KB
    DRAM lines) on the otherwise-idle GPSIMD queue.  DMA trigger
    DIRECT2D descriptor generation costs ~600-900ns of SEQUENCER time;
    four per-bt triggers on the sync/scalar queues were stalling the
    scalar sequencer in front of the EXPs and cost ~7us in the tail.
    All w16 loads also go via GPSIMD; w8/x8 on sync, x/wd on scalar.
    The last expert's final two tiles run as singleton groups (tile 14's
    eviction overlaps tile 15's matmuls) and split their writeback
    across sync+gpsimd, shortening the closing serial chain ~2us.

Startup (measured): engines boot ~6-8us; the ~7MB the PE needs (x16 3MB
+ x8 2MB + w8e0 1MB + w00 1MB) moves at ~0.23MB/us with all 16 DMA
engines saturated, so data is not fully in until ~38us.  The window is
filled with warm-up matmuls, the four routing-z chunks (each waiting on
its 512-col x chunk, split across both HWDGE queues), then expert 0.
~15us of PE idle remains and is provably DMA-byte-bound (reordering
x8/w8e0 ahead of x16 just moves the wait: the engines round-robin across
queues, so within-queue order is the only lever and the byte total is
fixed).  Tried and REVERTED as regressions: e0-before-routing (+8.6us),
per-bc routing-r (+4us), hyb-group-last (+2us), warm-fill between z
chunks (+2-7us, PSUM slot contention), bc-major 4KB-line x layout
(+3.4us).  The tile scheduler is chaotically sensitive to emission
order - measure before keeping ANY change here.

MEASUREMENT WARNING: the device intermittently enters multi-run clock
throttle episodes (EVERY engine uniformly ~1.2x slower: matmuls 250ns
instead of 213, EXP 1290 instead of 1087, exec ~345us instead of ~292).
Before attributing a +40-55us swing to a code change, check the matmul
duration histogram - if the steady-state bin moved 200-219 -> 240-259,
it is the chip, not the kernel.  Healthy-run band for this config:
291-294us.

Engine budget per fp8-class block: PE 8x213 = 1704ns; ScalarE 1087+182
= 1269ns; DVE recip+mul+stt ~= 1600ns (stt on cayman runs ~1.28us for
[128,1024] f16 - 2X_1PORT is the ceiling for tensor_tensor-class ops
and the silicon runs SBUF-source DVE/ACT ~2.3x below spec).  PE-bound.
"""

import numpy as np
import sys

for _p in ("/opt/trn_rl_repo", "/opt/pypackages"):
    if _p not in sys.path:
        sys.path.append(_p)

import concourse.bass as bass  # noqa: E402,F401
import concourse.bacc as bacc  # noqa: E402
import concourse.tile as tile  # noqa: E402
from concourse import mybir  # noqa: E402
from concourse.bass_utils import run_bass_kernel_spmd  # noqa: E402

TREE_DEPTH = 6
IN_F = 1024
OUT_F = 1024
BATCH = 2048
N_LEAVES = 64
N_NODES = 63
N_CORES = 8
E = N_LEAVES // N_CORES  # experts per core = 8
IC = IN_F // 128  # contraction chunks = 8
BT = BATCH // 128  # batch tiles = 16
F32 = mybir.dt.float32
BF16 = mybir.dt.bfloat16
F16 = mybir.dt.float16
F8E4 = mybir.dt.float8e4
DR = mybir.MatmulPerfMode.DoubleRow
AF = mybir.ActivationFunctionType
ALU = mybir.AluOpType

K8 = 512  # hybrid-class fp8 contraction half
KC8 = K8 // 256  # hybrid-class DoubleRow K-chunks = 2
KCF = IN_F // 256  # full-K DoubleRow K-chunks = 4
IC16 = (IN_F - K8) // 128  # hybrid-class f16 K-chunks = 4
HI_T = 2  # batch tiles (per core, post-permutation) on the hybrid path
SX, SW = 16.0, 64.0  # fp8 quantization scales for x and Wl
LOGIT_SCALE = 1.0 / (SX * SW)  # folded into the softmax Exp
K8R = 256  # routing fp8 K-range: z K[0:256] runs on the x8 data (1 DR mm)
ICR = (IN_F - K8R) // 128  # f16 x chunks = 6, covering K[256:1024]
SWD = 64.0  # fp8 scale for the Wd routing slice


def _tree_routes(depth):
    n_leaves = 2**depth
    idx = np.zeros((n_leaves, depth), dtype=np.int32)
    pos = np.zeros((n_leaves, depth), dtype=bool)
    for leaf in range(n_leaves):
        node, index = 0, leaf
        for d in range(depth):
            idx[leaf, d] = node
            pos[leaf, d] = index % 2 == 0
            node = node * 2 + 1 + index % 2
            index >>= 1
    return idx, pos


ROUTE_IDX, ROUTE_POS = _tree_routes(TREE_DEPTH)


def _selection_matrices():
    """A[n, l]=1 iff leaf l visits node n with the sigmoid branch; B for 1-sigmoid."""
    A = np.zeros((N_NODES, N_LEAVES), dtype=np.float32)
    B = np.zeros((N_NODES, N_LEAVES), dtype=np.float32)
    for leaf in range(N_LEAVES):
        for d in range(TREE_DEPTH):
            n = ROUTE_IDX[leaf, d]
            if ROUTE_POS[leaf, d]:
                A[n, leaf] = 1.0
            else:
                B[n, leaf] = 1.0
    return A, B


A_FULL, B_FULL = _selection_matrices()


def build_nc(add_bd: bool, add_bl: bool):
    """Build + compile the single-core Bass program (same NEFF on all cores)."""
    from contextlib import ExitStack

    MDT = F16

    nc = bacc.Bacc("TRN2", target_bir_lowering=False, debug=False)

    # host pre-arranges x/wd in SBUF tile order so one DMA per batch-chunk
    # moves 1KB-contiguous lines (small-line DMAs run at ~10GB/s and were
    # serializing the startup on the single hardware queue)
    xT_d = nc.dram_tensor("xT", [128, ICR, BATCH], MDT, kind="ExternalInput")
    wdT_d = nc.dram_tensor("wdT", [128, ICR, N_NODES], MDT, kind="ExternalInput")
    wd8T_d = nc.dram_tensor("wd8T", [128, 2, 64], F8E4, kind="ExternalInput")
    # f16 covers K 512..1023 of the hybrid-class contraction; the fp8
    # DoubleRow tensors cover the full K (k = kc*256 + i*128 + p).
    wl_d = nc.dram_tensor("wl", [E, IN_F - K8, OUT_F], MDT, kind="ExternalInput")
    x8T_d = nc.dram_tensor("x8T", [IN_F, BATCH], F8E4, kind="ExternalInput")
    wl8_d = nc.dram_tensor(
        "wl8", [E, KCF, 128, 2, OUT_F], F8E4, kind="ExternalInput"
    )
    a_d = nc.dram_tensor("amat", [N_NODES, E], F32, kind="ExternalInput")
    b_d = nc.dram_tensor("bmat", [N_NODES, E], F32, kind="ExternalInput")
    bd_d = bl_d = None
    if add_bd:
        bd_d = nc.dram_tensor("bd", [1, N_NODES], MDT, kind="ExternalInput")
    if add_bl:
        bl_d = nc.dram_tensor("bl", [E, OUT_F], MDT, kind="ExternalInput")
    out_d = nc.dram_tensor("out", [BATCH, OUT_F], F16, kind="ExternalOutput")

    mm = lambda out, lhsT, rhs, start, stop: nc.tensor.matmul(  # noqa: E731
        out, lhsT, rhs, start=start, stop=stop
    )

    with ExitStack() as ctx:
        tc = ctx.enter_context(tile.TileContext(nc))
        consts = ctx.enter_context(tc.tile_pool(name="consts", bufs=1))
        xp = ctx.enter_context(tc.tile_pool(name="xp", bufs=1))
        wp = ctx.enter_context(tc.tile_pool(name="wp", bufs=3))
        wp8 = ctx.enter_context(tc.tile_pool(name="wp8", bufs=3))
        accp = ctx.enter_context(tc.tile_pool(name="accp", bufs=BT))
        expp = ctx.enter_context(tc.tile_pool(name="expp", bufs=3))
        rpool = ctx.enter_context(tc.tile_pool(name="rpool", bufs=17))
        spp = ctx.enter_context(tc.tile_pool(name="spp", bufs=1))
        smallp = ctx.enter_context(tc.tile_pool(name="smallp", bufs=6))
        # ONE PSUM pool: 4 slots x [128,1024] f32 = all 8 banks.  Routing
        # z/r tiles are carved out of lp slots (partition/free subranges)
        # so the expert phase gets 4 live logit tiles - enough to overlap
        # a 2-batch-tile matmul group with the previous group's eviction.
        lps = ctx.enter_context(tc.tile_pool(name="lps", bufs=4, space="PSUM"))

        wd_t = consts.tile([128, ICR, N_NODES], MDT)
        # scalar HWDGE, first in its queue: wd gates the first routing MM
        nc.scalar.dma_start(out=wd_t, in_=wdT_d[:])
        wd8_t = consts.tile([128, 2, 64], F8E4)
        nc.scalar.dma_start(out=wd8_t, in_=wd8T_d[:])
        # -1/1024 per-partition scalar for the softplus(-z) stt (z PSUM is
        # at 1024x scale)
        nls_t = consts.tile([N_NODES, 1], F32)
        nc.vector.memset(nls_t, -LOGIT_SCALE)
        ab_t = consts.tile([2 * N_NODES, E], F32)
        nc.gpsimd.dma_start(out=ab_t[0:N_NODES, :], in_=a_d[:])
        nc.gpsimd.dma_start(out=ab_t[N_NODES : 2 * N_NODES, :], in_=b_d[:])
        ones_t = bd_t = None
        if add_bd or add_bl:
            ones_t = consts.tile([1, 512], MDT)
            nc.vector.memset(ones_t, 1.0)
        if add_bd:
            bd_t = consts.tile([1, N_NODES], MDT)
            nc.sync.dma_start(out=bd_t, in_=bd_d[:])

        def alloc_x():
            return xp.tile([128, ICR, BATCH], MDT, tag="x", name="x_t")

        def dma_x_bc(x_t, p):
            # each 512-column batch-chunk split across BOTH HWDGE queues
            # (K-chunks 0-3 on scalar, 4-7 on sync) so a chunk lands in
            # ~2.8us and routing is never starved long enough to cool HAM
            sl = slice(p * 512, (p + 1) * 512)
            nc.scalar.dma_start(
                out=x_t[:, 0 : ICR // 2, sl], in_=xT_d[:, 0 : ICR // 2, sl]
            )
            nc.sync.dma_start(
                out=x_t[:, ICR // 2 : ICR, sl], in_=xT_d[:, ICR // 2 : ICR, sl]
            )

        def dma_x(x_t):
            for p in range(BATCH // 512):
                dma_x_bc(x_t, p)

        def alloc_x8():
            return xp.tile([128, KCF, 2, BATCH], F8E4, tag="x8", bufs=1, name="x8")

        def dma_x8_kc(x8_t, kcs):
            # full-row pieces: 2KB contiguous lines (DMA engines are
            # per-line latency-bound; fatter lines -> more GB/s)
            for kc in kcs:
                for i in range(2):
                    r0 = kc * 256 + i * 128
                    nc.sync.dma_start(
                        out=x8_t[:, kc, i, :],
                        in_=x8T_d[r0 : r0 + 128, :],
                    )

        def dma_w(e, eng=None):
            eng = eng or nc.gpsimd
            w_t = wp.tile([128, IC16, OUT_F], MDT, tag="w", name=f"w_{e}")
            for c in range(IC16):
                # full-row: 2KB contiguous lines
                eng.dma_start(
                    out=w_t[:, c, :],
                    in_=wl_d[e, c * 128 : (c + 1) * 128, :],
                )
            return w_t

        def dma_w8_into(w8_t, e, kcs):
            for kc in kcs:
                # [128, 2, OUT_F] in one piece: 2KB contiguous lines on both
                # sides (wl8 host layout is [E, KCF, 128, 2, OUT_F])
                nc.sync.dma_start(
                    out=w8_t[:, kc, :, :],
                    in_=wl8_d[e, kc, :, :, :],
                )

        def dma_w8(e, kcs=None):
            w8_t = wp8.tile(
                [128, KCF, 2, OUT_F], F8E4, tag="w8", name=f"w8_{e}"
            )
            dma_w8_into(w8_t, e, kcs if kcs is not None else range(KCF))
            return w8_t

        r_ts = {}
        sp_ts = None

        def routing_zs(x_t, x8_t):
            nonlocal sp_ts
            # sp_stack rows 0:63 = softplus(-z) (written in place by the
            # DVE sub); rows 63:126 get softplus(z) via SBUF->SBUF DMA
            # (engines can cross partitions; compute engines cannot)
            sp_ts = (
                spp.tile([N_NODES, BATCH], F32, tag="sp_pos", name="sp_p"),
                spp.tile([2 * N_NODES, BATCH], F32, tag="sp_stack", name="sp_s"),
            )
            sp_pos, sp_stack = sp_ts
            NBC = BATCH // 512
            z_pss = []
            for bc in range(NBC):
                z_tile = lps.tile([128, OUT_F], F32, tag="lp", name="z_ps")
                z_ps = z_tile[0:N_NODES, 0:512]
                z_pss.append(z_ps)
                # K[0:256] on the fp8 path: one DoubleRow matmul against the
                # already-loaded x8 kc0 slice (saves 1MB of startup f16 x).
                # The dual-fp8 LDWEIGHTS needs an even stationary column
                # count, so wd8 is zero-padded to 64 nodes and this matmul
                # runs first (start=True defines row 63; Exp only reads 63).
                nc.tensor.matmul(
                    z_tile[0:64, 0:512],
                    wd8_t,
                    x8_t[:, 0, :, bc * 512 : (bc + 1) * 512],
                    start=True,
                    stop=False,
                    perf_mode=DR,
                )
                for c in range(ICR):
                    mm(
                        z_ps,
                        wd_t[:, c, :],
                        x_t[:, c, bc * 512 : (bc + 1) * 512],
                        start=False,
                        stop=(c == ICR - 1 and not add_bd),
                    )
                if add_bd:
                    mm(z_ps, bd_t[0:1, :], ones_t[0:1, :], start=False, stop=True)
            # all Exps, then all Lns: the scalar engine isn't thrashing the
            # Exp<->Ln ACT table set (each swap costs ~1.3us)
            for bc in range(NBC):
                sl = slice(bc * 512, (bc + 1) * 512)
                nc.scalar.activation(
                    sp_pos[:, sl], z_pss[bc], AF.Exp, scale=LOGIT_SCALE
                )
            for bc in range(NBC):
                sl = slice(bc * 512, (bc + 1) * 512)
                nc.scalar.activation(sp_pos[:, sl], sp_pos[:, sl], AF.Ln, bias=1.0)
            for bc in range(NBC):
                sl = slice(bc * 512, (bc + 1) * 512)
                # softplus(-z) = softplus(z) - z (exact; z PSUM is at 1024x,
                # so fold -1/1024 in via the stt scalar); DVE is idle here
                nc.vector.scalar_tensor_tensor(
                    sp_stack[0:N_NODES, sl], z_pss[bc], nls_t, sp_pos[:, sl],
                    op0=ALU.mult, op1=ALU.add,
                )
                # stack softplus(z) under it while the DMA engines have slack
                nc.gpsimd.dma_start(
                    out=sp_stack[N_NODES : 2 * N_NODES, sl], in_=sp_pos[:, sl]
                )

        def routing_r():
            _, sp_stack = sp_ts
            for bt in range(BT):
                r_ps = lps.tile([128, OUT_F], F32, tag="lp", name="r_ps")[:, 0:E]
                nc.tensor.matmul(
                    r_ps,
                    sp_stack[:, bt * 128 : (bt + 1) * 128],
                    ab_t,
                    start=True,
                    stop=True,
                )
                r_t = rpool.tile([128, E], F32, tag="r", name=f"r_{bt}")
                nc.scalar.activation(r_t, r_ps, AF.Exp, scale=-1.0)
                r_ts[bt] = r_t

        s0_ts = {}
        ble_ts = {}

        def expert(e, x_t, w_t, accs, x8_t, w8_t, bts=None):
            ble_t = None
            if add_bl:
                if e not in ble_ts:
                    ble_t = smallp.tile([1, OUT_F], MDT, tag="ble", bufs=2)
                    nc.sync.dma_start(out=ble_t, in_=bl_d[e : e + 1, :])
                    ble_ts[e] = ble_t
                ble_t = ble_ts[e]
            groups = bts if bts is not None else [
                (bt, bt + 1) for bt in range(0, BT, 2)
            ]
            if e == E - 1 and bts is None:
                # singleton final groups: tile 14's eviction chain overlaps
                # tile 15's matmuls instead of serializing after them
                groups = groups[:-1] + [(BT - 2,), (BT - 1,)]
            for btg in groups:
                hyb = btg[0] < HI_T
                # stationary-reuse order: kc-outer, bt-inner, so each
                # 256-row DR weight load streams two 512-col passes (a DR
                # load costs ~40ns over the 213ns stream when fresh per
                # matmul; amortized over a pair it hides completely)
                lp = {bt: lps.tile([128, OUT_F], F32, tag="lp", name=f"lp_{bt}") for bt in btg}
                bsl = {bt: slice(bt * 128, (bt + 1) * 128) for bt in btg}
                for oc in range(2):
                    osl = slice(oc * 512, (oc + 1) * 512)
                    if hyb:
                        for kc in range(KC8):
                            for bt in btg:
                                nc.tensor.matmul(
                                    lp[bt][:, osl],
                                    x8_t[:, kc, :, bsl[bt]],
                                    w8_t[:, kc, :, osl],
                                    start=(kc == 0),
                                    stop=False,
                                    perf_mode=DR,
                                )
                        for c in range(IC16):
                            for bt in btg:
                                mm(
                                    lp[bt][:, osl],
                                    x_t[:, ICR - IC16 + c, bsl[bt]],
                                    w_t[:, c, osl],
                                    start=False,
                                    stop=(c == IC16 - 1 and not add_bl),
                                )
                    else:
                        for kc in range(KCF):
                            for bt in btg:
                                nc.tensor.matmul(
                                    lp[bt][:, osl],
                                    x8_t[:, kc, :, bsl[bt]],
                                    w8_t[:, kc, :, osl],
                                    start=(kc == 0),
                                    stop=(kc == KCF - 1 and not add_bl),
                                    perf_mode=DR,
                                )
                    if add_bl:
                        for bt in btg:
                            mm(
                                lp[bt][:, osl],
                                ones_t[0:1, 0:128],
                                ble_t[0:1, osl],
                                start=False,
                                stop=True,
                            )
                for bt in btg:
                    target = accs[bt] if e == 0 else None
                    if e != 0:
                        # f16 exp values: halves ScalarE write + DVE read
                        # traffic; quantization (5e-4 rel on ~unit values) is
                        # negligible next to the fp8 noise budget
                        target = expp.tile(
                            [128, OUT_F], F16, tag="exp", name="exp_t"
                        )
                    sh = smallp.tile(
                        [128, 1], F32,
                        tag=("s0h" if e == 0 else "sh"),
                        bufs=(BT + 2 if e == 0 else 6),
                        name=f"sh_{e == 0}",
                    )
                    # ONE fused activation (exp+accum over both 512-col
                    # halves of the two-bank PSUM tile) evicts the block
                    nc.scalar.activation(
                        target, lp[bt], AF.Exp, scale=LOGIT_SCALE, accum_out=sh
                    )
                    if e == 0:
                        # Pure-ScalarE eviction: park unscaled exp in the f16
                        # accumulator; the routing/softmax scale is applied at
                        # e==1, so expert 0 needs no routing result.
                        s0_ts[bt] = sh
                        continue
                    if e == 1:
                        # deferred scale of expert 0's parked contribution
                        d_t = smallp.tile([128, 1], F32, tag="d")
                        nc.vector.reciprocal(d_t, s0_ts[bt])
                        d2_t = smallp.tile([128, 1], F32, tag="d2")
                        nc.vector.tensor_scalar_mul(
                            d2_t, d_t, r_ts[bt][:, 0:1]
                        )
                        nc.vector.tensor_scalar_mul(accs[bt], accs[bt], d2_t)
                    exp_t = target
                    sc_t = smallp.tile([128, 1], F32, tag="sc")
                    nc.vector.reciprocal(sc_t, sh)
                    sc2_t = smallp.tile([128, 1], F32, tag="sc2")
                    nc.vector.tensor_scalar_mul(
                        sc2_t, sc_t, r_ts[bt][:, e : e + 1]
                    )
                    nc.vector.scalar_tensor_tensor(
                        accs[bt], exp_t, sc2_t, accs[bt],
                        op0=ALU.mult, op1=ALU.add,
                    )
                    if e == E - 1:
                        # single full-width writeback (2KB DRAM lines) on the
                        # otherwise-idle gpsimd queue: DIRECT2D descriptor
                        # generation (~600ns/trigger) was stalling the scalar
                        # sequencer in front of the EXPs
                        r0 = bt * 128
                        if bt >= BT - 2:
                            # final tiles: halve the closing DMA by splitting
                            # across the (now idle) sync queue and gpsimd
                            nc.sync.dma_start(
                                out=out_d[r0 : r0 + 128, 0:512],
                                in_=accs[bt][:, 0:512],
                            )
                            nc.gpsimd.dma_start(
                                out=out_d[r0 : r0 + 128, 512:OUT_F],
                                in_=accs[bt][:, 512:OUT_F],
                            )
                        else:
                            nc.gpsimd.dma_start(
                                out=out_d[r0 : r0 + 128, :], in_=accs[bt]
                            )

        def alloc_accs():
            return [
                accp.tile([128, OUT_F], F16, tag="acc", name=f"acc_{bt}")
                for bt in range(BT)
            ]

        # ---- PE warm-up: ~4.5us of throwaway matmuls on a memset tile (no
        # DMA dependency, starts immediately) flips the HAM clock gate to
        # 8/8 before the real Z matmuls start, and fills the PE-idle window
        # while x streams in.
        warm_w = consts.tile([128, 8 * N_NODES], MDT)
        nc.vector.memset(warm_w, 1.0)
        warm_ps = lps.tile([128, OUT_F], F32, tag="lp", name="warm_ps")[0:N_NODES, 0 : 7 * N_NODES]
        NWARM = 12
        for i in range(NWARM):
            mm(
                warm_ps,
                warm_w[:, 0:N_NODES],
                warm_w[:, N_NODES : 8 * N_NODES],
                start=(i == 0),
                stop=(i == NWARM - 1),
            )
        junk_t = smallp.tile([N_NODES, 1], F32, tag="junk")
        nc.vector.reduce_max(junk_t, warm_ps, axis=mybir.AxisListType.X)

        # ---- emission order tuned so the PE never waits after startup ----
        # The startup is DMA-bound (~8MB must land before expert 1 can
        # stream full-rate), so routing z-chunks and expert-0 blocks
        # interleave, each emitted right after its own DMA pieces: the PE
        # starts on routing bc0 after ~1.1MB and never idles long enough
        # for the HAM clock gate to re-throttle.  Expert 0 needs no
        # routing result (it parks unscaled exp in the accumulator), so it
        # can run before routing completes.
        # queue FIFOs: scalar = [wd, x bc0, x bc2, w16_0];
        # sync = [x bc1, x bc3, x8 kc0-1, w8_0 kc0-1, x8 kc2-3, w8_0 kc2-3,
        #         w8_1, ...]
        x0 = alloc_x()
        x8 = alloc_x8()
        dma_x8_kc(x8, [0])  # 0.25MB, gates the 4 routing DR matmuls
        dma_x(x0)
        dma_x8_kc(x8, [1])
        w800 = dma_w8(0, kcs=range(KC8))
        dma_x8_kc(x8, range(KC8, KCF))
        dma_w8_into(w800, 0, range(KC8, KCF))
        w00 = dma_w(0)  # gpsimd queue
        accs0 = alloc_accs()
        routing_zs(x0, x8)
        expert(0, x0, w00, accs0, x8, w800)
        # routing-r runs AFTER expert 0 (only expert 1+ needs r): off the
        # DMA-saturated startup window, and merged to ONE matmul per batch
        # tile ([sp_neg; sp_pos] stacked on 126 partitions against [A; B])
        routing_r()
        for e in range(1, E):
            expert(e, x0, dma_w(e), accs0, x8, dma_w8(e))

    nc.compile()
    return nc


def _routing_host(x, Wd, bd):
    """Replicate the device routing numerics (fp8 K[0:256] + f16 rest)."""
    import ml_dtypes

    f8 = ml_dtypes.float8_e4m3fn
    x8r = np.clip(x[:, :K8R] * SX, -240, 240).astype(f8).astype(np.float32)
    wd8 = np.clip(Wd[:, :K8R] * SWD, -240, 240).astype(f8).astype(np.float32)
    x16 = x[:, K8R:].astype(np.float16).astype(np.float32)
    wd16 = (Wd[:, K8R:] * (SX * SWD)).astype(np.float16).astype(np.float32)
    z = (x8r @ wd8.T + x16 @ wd16.T) * (1.0 / (SX * SWD))
    if bd is not None and np.any(bd):
        z = z + bd.astype(np.float16).astype(np.float32)
    sp = np.log(np.exp(z) + 1.0)  # softplus(z)
    logr = -((sp - z) @ A_FULL + sp @ B_FULL)
    return np.exp(logr)  # [B, 64]


def make_inputs(x, Wd, bd, Wl, bl, add_bd, add_bl):
    """Per-core input maps + row permutations (adaptive precision classes)."""
    import ml_dtypes

    ndt = np.float16
    f8 = ml_dtypes.float8_e4m3fn

    r = _routing_host(x, Wd, bd)
    # rank rows per core by max leaf routing weight; top HI_T*128 rows get
    # the hybrid (lower-noise) path
    metric = r.reshape(BATCH, N_CORES, E).max(axis=2)  # [B, cores]

    wdT = (
        (Wd[:, K8R:] * (SX * SWD)).T.astype(ndt)
        .reshape(ICR, 128, N_NODES)
        .transpose(1, 0, 2)
    )
    wdT = np.ascontiguousarray(wdT)
    # wd8T[p, i, n] = fp8(SWD * Wd[n, i*128 + p]), zero-padded to 64 nodes
    wd8p = np.zeros((64, K8R), np.float32)
    wd8p[:N_NODES] = np.clip(Wd[:, :K8R] * SWD, -240, 240)
    wd8T = np.ascontiguousarray(
        wd8p.astype(f8).T.reshape(2, 128, 64).transpose(1, 0, 2)
    )
    x8_full = np.clip(x * SX, -240, 240).astype(f8)  # [B, K]

    in_maps, perms = [], []
    for core in range(N_CORES):
        lo, hi = core * E, (core + 1) * E
        perm = np.argsort(-metric[:, core], kind="stable")
        xg = x[perm]
        # SBUF-tile-order layouts: [128, IC, cols] with 1KB contiguous lines
        xT = xg.T[K8R:].astype(ndt).reshape(ICR, 128, BATCH).transpose(1, 0, 2)
        # [K, B]: x8T[k, b] = fp8(SX * xg[b, k]) ; device views rows
        # k = kc*256 + i*128 + p in [128, KCF, 2, B] tile order
        x8T = x8_full[perm].T
        m = {
            "xT": np.ascontiguousarray(xT),
            "wdT": wdT,
            "wd8T": wd8T,
            "x8T": np.ascontiguousarray(x8T),
            "amat": np.ascontiguousarray(A_FULL[:, lo:hi]),
            "bmat": np.ascontiguousarray(B_FULL[:, lo:hi]),
            # f16 half carries 1024*W so both paths land at 1024x logit scale
            "wl": np.ascontiguousarray(
                (Wl[lo:hi, :, K8:] * (SX * SW)).transpose(0, 2, 1).astype(ndt)
            ),
        }
        # [E, KCF, 128, 2, OUT_F]: w8[e,kc,p,i,o] = fp8(SW*W[e,o,kc*256+i*128+p])
        w8 = np.clip(Wl[lo:hi] * SW, -240, 240).astype(f8)
        m["wl8"] = np.ascontiguousarray(
            w8.transpose(0, 2, 1)
            .reshape(E, KCF, 2, 128, OUT_F)
            .transpose(0, 1, 3, 2, 4)
        )
        if add_bd:
            m["bd"] = np.ascontiguousarray(bd.astype(ndt)).reshape(1, N_NODES)
        if add_bl:
            m["bl"] = np.ascontiguousarray((bl[lo:hi] * (SX * SW)).astype(ndt))
        in_maps.append(m)
        perms.append(perm)
    return in_maps, perms


_NC_CACHE = {}


def _get_nc(add_bd, add_bl):
    key = (add_bd, add_bl)
    if key not in _NC_CACHE:
        _NC_CACHE[key] = build_nc(add_bd, add_bl)
    return _NC_CACHE[key]


def run_spmd(x, Wd, bd, Wl, bl, trace=False):
    add_bd = bool(np.any(bd))
    add_bl = bool(np.any(bl))
    nc = _get_nc(add_bd, add_bl)
    in_maps, perms = make_inputs(x, Wd, bd, Wl, bl, add_bd, add_bl)
    res = run_bass_kernel_spmd(nc, in_maps, core_ids=list(range(N_CORES)), trace=trace)
    out = np.zeros((BATCH, OUT_F), np.float64)
    for core, r in enumerate(res.results):
        part = np.empty((BATCH, OUT_F), np.float32)
        part[perms[core]] = r["out"].astype(np.float32)
        out += part
    return out.astype(np.float32), res


def kernel(x, Wd, bd, Wl, bl):
    out, _ = run_spmd(
        np.asarray(x), np.asarray(Wd), np.asarray(bd), np.asarray(Wl), np.asarray(bl)
    )
    return out


# revision 41
# speedup vs baseline: 1.0024x; 1.0024x over previous
"""DNDF tree (soft decision tree / dense MoE) kernel for Trainium2.

Full computation (reference):
    dprob  = sigmoid(x @ Wd.T + bd)                 [B, 63]
    routing[b, l] = prod_d (pos ? dprob[idx] : 1 - dprob[idx])   [B, 64]
    leaves = softmax(einsum('bi,loi->blo', x, Wl) + bl, axis=-1) [B, 64, O]
    out    = einsum('bl,blo->bo', routing, leaves)  [B, O]

Sharding: expert-parallel over the 64 leaves: core c owns leaves
8c..8c+7, computes partial = sum_{l in core} routing[:, l] * leaves[:, l, :]
over the FULL batch; the host sums the 8 per-core partials.

Routing trick (no gathers): with z = x@Wd.T + bd,
    log p      = -softplus(-z),   log(1-p) = -softplus(z)
    log routing[b,l] = -( [softplus(-z); softplus(z)] @ [A; B] )[b, l]
where A[n,l]=1 iff leaf l visits node n on the sigmoid branch and B for
the (1-sigmoid) branch - ONE 126-K matmul per batch tile against the
host-stacked [A; B] ([softplus(-z) is computed in place in rows 0:63 of
the stacked tile; softplus(z) is DMA-copied into rows 63:126 - compute
engines cannot cross partitions but DMA can).  softplus(t) = Ln(Exp(t)+1)
with the one ACT table set that has both exp and ln; all four Exp chunks
run before the four Ln chunks so the table set is not swapped (~1.3us)
per chunk.  routing-r runs AFTER expert 0 (only experts 1+ consume r),
off the DMA-saturated startup window.

Routing z runs K[0:256] on the fp8 path (one DoubleRow matmul against
the x8 tile's kc0 slice; wd8 zero-padded to 64 nodes - the dual-fp8
LDWEIGHTS ISA check rejects an odd stationary column count) and
K[256:1024] in f16.  This trims the startup-critical f16 x tensor from
4MB to 3MB (the window is DMA-byte-bound) and costs 1.64e-2 -> 1.77e-2
rel err (sim 1.743e-2; full half-K fp8 routing measured 2.07e-2, over
the gate).  The z PSUM is then at 1024x scale: the softplus Exp folds
in 1/1024 and the softplus(-z) subtraction becomes a DVE stt with a
-1/1024 per-partition scalar.

Adaptive per-(core, batch-row) precision ("ada8"): the output error a
core's fp8 noise causes at a row scales with that core's routing weight
there, and the max-err statistic is dominated by a handful of rows where
one leaf's routing weight is large.  So the host ranks rows per core by
max leaf routing weight (a cheap [B,63] GEMM + softplus in numpy),
permutes the batch per core so the top-256 rows land in batch tiles 0-1,
and the device runs:
  - tiles 0-1 ("hyb512"):  K 0..511 fp8e4 DoubleRow + K 512..1023 f16
  - tiles 2-15 ("fp8"):    all 1024 K as 4 fp8 DoubleRow matmuls
That is 1088 PE matmuls per core instead of 1536 (-29%).  Measured HW
rel err 1.638e-2 (numpy sim of the full quantization chain predicted
1.640e-2; the sim tracks HW within ~2%).  Uniform K8=768 (2.17e-2),
mixing per *expert* (2.2e-2+), fp8 routing (2.07e-2 even half-K), and
only 1 protected tile (2.2e-2) all breach the 2e-2 gate - per-row
targeting with 2 protected tiles is what keeps the max error pinned.
f16 accumulators add 4e-4 (1.602->1.640 sim) and halve DVE/SBUF cost.

Scales: x8 = e4m3(16x), w8 = e4m3(64w), w16 = f16(1024w) so all paths
accumulate logits at 1024x; the softmax exp folds in scale=1/1024.

PE stream discipline (the big wins, measured):
  - Stationary reuse: kc-outer/bt-inner matmul order within a 2-tile
    group streams each 256-row DR weight load across two 512-col passes;
    a fresh-per-matmul DR load costs +40-60ns over the 213ns stream
    (272ns avg -> 216ns avg, -56us).  Needs 4 live [128,1024] PSUM logit
    tiles = all 8 banks, so routing z/r tiles are carved from the same
    single PSUM pool (warm-up included - a stale handle would alias).
  - ONE fused ScalarE activation per block (exp over both 512-col halves
    of the 2-bank PSUM tile, accum_out for the softmax denominator):
    (1024+352)/1.2 ~= 1.1us, vs 2x720 + extra READ_ACCUM unfused.
  - Expert 0 parks unscaled exp in the f16 accumulator via its eviction
    activation (pure ScalarE); expert 1 applies the deferred scale, so
    expert 0 can run before routing-r exists.
  - Writeback: one full-width [128,1024] f16 DMA per batch tile (2KB
    DRAM lines) on the otherwise-idle GPSIMD queue.  DMA trigger
    DIRECT2D descriptor generation costs ~600-900ns of SEQUENCER time;
    four per-bt triggers on the sync/scalar queues were stalling the
    scalar sequencer in front of the EXPs and cost ~7us in the tail.
    All w16 loads also go via GPSIMD; w8/x8 on sync, x/wd on scalar.
    The last expert's final two tiles run as singleton groups (tile 14's
    eviction overlaps tile 15's matmuls) and split their writeback
    across sync+gpsimd, shortening the closing serial chain ~2us.

Startup (measured): engines boot ~6-8us; the ~7MB the PE needs (x16 3MB
+ x8 2MB + w8e0 1MB + w00 1MB) moves at ~0.23MB/us with all 16 DMA
engines saturated, so data is not fully in until ~38us.  The window is
filled with warm-up matmuls, the four routing-z chunks (each waiting on
its 512-col x chunk, split across both HWDGE queues), then expert 0.
~15us of PE idle remains and is provably DMA-byte-bound (reordering
x8/w8e0 ahead of x16 just moves the wait: the engines round-robin across
queues, so within-queue order is the only lever and the byte total is
fixed).  Tried and REVERTED as regressions: e0-before-routing (+8.6us),
per-bc routing-r (+4us), hyb-group-last (+2us), warm-fill between z
chunks (+2-7us, PSUM slot contention), bc-major 4KB-line x layout
(+3.4us).  The tile scheduler is chaotically sensitive to emission
order - measure before keeping ANY change here.

MEASUREMENT WARNING: the device intermittently enters multi-run clock
throttle episodes (EVERY engine uniformly ~1.2x slower: matmuls 250ns
instead of 213, EXP 1290 instead of 1087, exec ~345us instead of ~292).
Before attributing a +40-55us swing to a code change, check the matmul
duration histogram - if the steady-state bin moved 200-219 -> 240-259,
it is the chip, not the kernel.  Healthy-run band for this config:
291-294us.

Engine budget per fp8-class block: PE 8x213 = 1704ns; ScalarE 1087+182
= 1269ns; DVE recip+mul+stt ~= 1600ns (stt on cayman runs ~1.28us for
[128,1024] f16 - 2X_1PORT is the ceiling for tensor_tensor-class ops
and the silicon runs SBUF-source DVE/ACT ~2.3x below spec).  PE-bound.
"""

import numpy as np
import sys

for _p in ("/opt/trn_rl_repo", "/opt/pypackages"):
    if _p not in sys.path:
        sys.path.append(_p)

import concourse.bass as bass  # noqa: E402,F401
import concourse.bacc as bacc  # noqa: E402
import concourse.tile as tile  # noqa: E402
from concourse import mybir  # noqa: E402
from concourse.bass_utils import run_bass_kernel_spmd  # noqa: E402

TREE_DEPTH = 6
IN_F = 1024
OUT_F = 1024
BATCH = 2048
N_LEAVES = 64
N_NODES = 63
N_CORES = 8
E = N_LEAVES // N_CORES  # experts per core = 8
IC = IN_F // 128  # contraction chunks = 8
BT = BATCH // 128  # batch tiles = 16
F32 = mybir.dt.float32
BF16 = mybir.dt.bfloat16
F16 = mybir.dt.float16
F8E4 = mybir.dt.float8e4
DR = mybir.MatmulPerfMode.DoubleRow
AF = mybir.ActivationFunctionType
ALU = mybir.AluOpType

K8 = 512  # hybrid-class fp8 contraction half
KC8 = K8 // 256  # hybrid-class DoubleRow K-chunks = 2
KCF = IN_F // 256  # full-K DoubleRow K-chunks = 4
IC16 = (IN_F - K8) // 128  # hybrid-class f16 K-chunks = 4
HI_T = 2  # batch tiles (per core, post-permutation) on the hybrid path
SX, SW = 16.0, 64.0  # fp8 quantization scales for x and Wl
LOGIT_SCALE = 1.0 / (SX * SW)  # folded into the softmax Exp
K8R = 256  # routing fp8 K-range: z K[0:256] runs on the x8 data (1 DR mm)
ICR = (IN_F - K8R) // 128  # f16 x chunks = 6, covering K[256:1024]
SWD = 64.0  # fp8 scale for the Wd routing slice


def _tree_routes(depth):
    n_leaves = 2**depth
    idx = np.zeros((n_leaves, depth), dtype=np.int32)
    pos = np.zeros((n_leaves, depth), dtype=bool)
    for leaf in range(n_leaves):
        node, index = 0, leaf
        for d in range(depth):
            idx[leaf, d] = node
            pos[leaf, d] = index % 2 == 0
            node = node * 2 + 1 + index % 2
            index >>= 1
    return idx, pos


ROUTE_IDX, ROUTE_POS = _tree_routes(TREE_DEPTH)


def _selection_matrices():
    """A[n, l]=1 iff leaf l visits node n with the sigmoid branch; B for 1-sigmoid."""
    A = np.zeros((N_NODES, N_LEAVES), dtype=np.float32)
    B = np.zeros((N_NODES, N_LEAVES), dtype=np.float32)
    for leaf in range(N_LEAVES):
        for d in range(TREE_DEPTH):
            n = ROUTE_IDX[leaf, d]
            if ROUTE_POS[leaf, d]:
                A[n, leaf] = 1.0
            else:
                B[n, leaf] = 1.0
    return A, B


A_FULL, B_FULL = _selection_matrices()


def build_nc(add_bd: bool, add_bl: bool):
    """Build + compile the single-core Bass program (same NEFF on all cores)."""
    from contextlib import ExitStack

    MDT = F16

    nc = bacc.Bacc("TRN2", target_bir_lowering=False, debug=False)

    # host pre-arranges x/wd in SBUF tile order so one DMA per batch-chunk
    # moves 1KB-contiguous lines (small-line DMAs run at ~10GB/s and were
    # serializing the startup on the single hardware queue)
    xT_d = nc.dram_tensor("xT", [128, ICR, BATCH], MDT, kind="ExternalInput")
    wdT_d = nc.dram_tensor("wdT", [128, ICR, N_NODES], MDT, kind="ExternalInput")
    wd8T_d = nc.dram_tensor("wd8T", [128, 2, 64], F8E4, kind="ExternalInput")
    # f16 covers K 512..1023 of the hybrid-class contraction; the fp8
    # DoubleRow tensors cover the full K (k = kc*256 + i*128 + p).
    wl_d = nc.dram_tensor("wl", [E, IN_F - K8, OUT_F], MDT, kind="ExternalInput")
    x8T_d = nc.dram_tensor("x8T", [IN_F, BATCH], F8E4, kind="ExternalInput")
    wl8_d = nc.dram_tensor(
        "wl8", [E, KCF, 128, 2, OUT_F], F8E4, kind="ExternalInput"
    )
    a_d = nc.dram_tensor("amat", [N_NODES, E], F32, kind="ExternalInput")
    b_d = nc.dram_tensor("bmat", [N_NODES, E], F32, kind="ExternalInput")
    bd_d = bl_d = None
    if add_bd:
        bd_d = nc.dram_tensor("bd", [1, N_NODES], MDT, kind="ExternalInput")
    if add_bl:
        bl_d = nc.dram_tensor("bl", [E, OUT_F], MDT, kind="ExternalInput")
    out_d = nc.dram_tensor("out", [BATCH, OUT_F], F16, kind="ExternalOutput")

    mm = lambda out, lhsT, rhs, start, stop: nc.tensor.matmul(  # noqa: E731
        out, lhsT, rhs, start=start, stop=stop
    )

    with ExitStack() as ctx:
        tc = ctx.enter_context(tile.TileContext(nc))
        consts = ctx.enter_context(tc.tile_pool(name="consts", bufs=1))
        xp = ctx.enter_context(tc.tile_pool(name="xp", bufs=1))
        wp = ctx.enter_context(tc.tile_pool(name="wp", bufs=3))
        wp8 = ctx.enter_context(tc.tile_pool(name="wp8", bufs=3))
        accp = ctx.enter_context(tc.tile_pool(name="accp", bufs=BT))
        expp = ctx.enter_context(tc.tile_pool(name="expp", bufs=3))
        rpool = ctx.enter_context(tc.tile_pool(name="rpool", bufs=17))
        spp = ctx.enter_context(tc.tile_pool(name="spp", bufs=1))
        smallp = ctx.enter_context(tc.tile_pool(name="smallp", bufs=6))
        # ONE PSUM pool: 4 slots x [128,1024] f32 = all 8 banks.  Routing
        # z/r tiles are carved out of lp slots (partition/free subranges)
        # so the expert phase gets 4 live logit tiles - enough to overlap
        # a 2-batch-tile matmul group with the previous group's eviction.
        lps = ctx.enter_context(tc.tile_pool(name="lps", bufs=4, space="PSUM"))

        wd_t = consts.tile([128, ICR, N_NODES], MDT)
        # scalar HWDGE, first in its queue: wd gates the first routing MM
        nc.scalar.dma_start(out=wd_t, in_=wdT_d[:])
        wd8_t = consts.tile([128, 2, 64], F8E4)
        nc.scalar.dma_start(out=wd8_t, in_=wd8T_d[:])
        # -1/1024 per-partition scalar for the softplus(-z) stt (z PSUM is
        # at 1024x scale)
        nls_t = consts.tile([N_NODES, 1], F32)
        nc.vector.memset(nls_t, -LOGIT_SCALE)
        ab_t = consts.tile([2 * N_NODES, E], F32)
        nc.gpsimd.dma_start(out=ab_t[0:N_NODES, :], in_=a_d[:])
        nc.gpsimd.dma_start(out=ab_t[N_NODES : 2 * N_NODES, :], in_=b_d[:])
        ones_t = bd_t = None
        if add_bd or add_bl:
            ones_t = consts.tile([1, 512], MDT)
            nc.vector.memset(ones_t, 1.0)
        if add_bd:
            bd_t = consts.tile([1, N_NODES], MDT)
            nc.sync.dma_start(out=bd_t, in_=bd_d[:])

        def alloc_x():
            return xp.tile([128, ICR, BATCH], MDT, tag="x", name="x_t")

        def dma_x_bc(x_t, p):
            # each 512-column batch-chunk split across BOTH HWDGE queues
            # (K-chunks 0-3 on scalar, 4-7 on sync) so a chunk lands in
            # ~2.8us and routing is never starved long enough to cool HAM
            sl = slice(p * 512, (p + 1) * 512)
            nc.scalar.dma_start(
                out=x_t[:, 0 : ICR // 2, sl], in_=xT_d[:, 0 : ICR // 2, sl]
            )
            nc.sync.dma_start(
                out=x_t[:, ICR // 2 : ICR, sl], in_=xT_d[:, ICR // 2 : ICR, sl]
            )

        def dma_x(x_t):
            for p in range(BATCH // 512):
                dma_x_bc(x_t, p)

        def alloc_x8():
            return xp.tile([128, KCF, 2, BATCH], F8E4, tag="x8", bufs=1, name="x8")

        def dma_x8_kc(x8_t, kcs):
            # full-row pieces: 2KB contiguous lines (DMA engines are
            # per-line latency-bound; fatter lines -> more GB/s)
            for kc in kcs:
                for i in range(2):
                    r0 = kc * 256 + i * 128
                    nc.sync.dma_start(
                        out=x8_t[:, kc, i, :],
                        in_=x8T_d[r0 : r0 + 128, :],
                    )

        def dma_w(e, eng=None):
            eng = eng or nc.gpsimd
            w_t = wp.tile([128, IC16, OUT_F], MDT, tag="w", name=f"w_{e}")
            for c in range(IC16):
                # full-row: 2KB contiguous lines
                eng.dma_start(
                    out=w_t[:, c, :],
                    in_=wl_d[e, c * 128 : (c + 1) * 128, :],
                )
            return w_t

        def dma_w8_into(w8_t, e, kcs):
            for kc in kcs:
                # [128, 2, OUT_F] in one piece: 2KB contiguous lines on both
                # sides (wl8 host layout is [E, KCF, 128, 2, OUT_F])
                nc.sync.dma_start(
                    out=w8_t[:, kc, :, :],
                    in_=wl8_d[e, kc, :, :, :],
                )

        def dma_w8(e, kcs=None):
            w8_t = wp8.tile(
                [128, KCF, 2, OUT_F], F8E4, tag="w8", name=f"w8_{e}"
            )
            dma_w8_into(w8_t, e, kcs if kcs is not None else range(KCF))
            return w8_t

        r_ts = {}
        sp_ts = None

        def routing_zs(x_t, x8_t):
            nonlocal sp_ts
            # sp_stack rows 0:63 = softplus(-z) (written in place by the
            # DVE sub); rows 63:126 get softplus(z) via SBUF->SBUF DMA
            # (engines can cross partitions; compute engines cannot)
            sp_ts = (
                spp.tile([N_NODES, BATCH], F32, tag="sp_pos", name="sp_p"),
                spp.tile([2 * N_NODES, BATCH], F32, tag="sp_stack", name="sp_s"),
            )
            sp_pos, sp_stack = sp_ts
            NBC = BATCH // 512
            z_pss = []
            for bc in range(NBC):
                z_tile = lps.tile([128, OUT_F], F32, tag="lp", name="z_ps")
                z_ps = z_tile[0:N_NODES, 0:512]
                z_pss.append(z_ps)
                # K[0:256] on the fp8 path: one DoubleRow matmul against the
                # already-loaded x8 kc0 slice (saves 1MB of startup f16 x).
                # The dual-fp8 LDWEIGHTS needs an even stationary column
                # count, so wd8 is zero-padded to 64 nodes and this matmul
                # runs first (start=True defines row 63; Exp only reads 63).
                nc.tensor.matmul(
                    z_tile[0:64, 0:512],
                    wd8_t,
                    x8_t[:, 0, :, bc * 512 : (bc + 1) * 512],
                    start=True,
                    stop=False,
                    perf_mode=DR,
                )
                for c in range(ICR):
                    mm(
                        z_ps,
                        wd_t[:, c, :],
                        x_t[:, c, bc * 512 : (bc + 1) * 512],
                        start=False,
                        stop=(c == ICR - 1 and not add_bd),
                    )
                if add_bd:
                    mm(z_ps, bd_t[0:1, :], ones_t[0:1, :], start=False, stop=True)
            # all Exps, then all Lns: the scalar engine isn't thrashing the
            # Exp<->Ln ACT table set (each swap costs ~1.3us)
            for bc in range(NBC):
                sl = slice(bc * 512, (bc + 1) * 512)
                nc.scalar.activation(
                    sp_pos[:, sl], z_pss[bc], AF.Exp, scale=LOGIT_SCALE
                )
            for bc in range(NBC):
                sl = slice(bc * 512, (bc + 1) * 512)
                nc.scalar.activation(sp_pos[:, sl], sp_pos[:, sl], AF.Ln, bias=1.0)
            for bc in range(NBC):
                sl = slice(bc * 512, (bc + 1) * 512)
                # softplus(-z) = softplus(z) - z (exact; z PSUM is at 1024x,
                # so fold -1/1024 in via the stt scalar); DVE is idle here
                nc.vector.scalar_tensor_tensor(
                    sp_stack[0:N_NODES, sl], z_pss[bc], nls_t, sp_pos[:, sl],
                    op0=ALU.mult, op1=ALU.add,
                )
                # stack softplus(z) under it while the DMA engines have slack
                nc.gpsimd.dma_start(
                    out=sp_stack[N_NODES : 2 * N_NODES, sl], in_=sp_pos[:, sl]
                )

        def routing_r():
            _, sp_stack = sp_ts
            for bt in range(BT):
                r_ps = lps.tile([128, OUT_F], F32, tag="lp", name="r_ps")[:, 0:E]
                nc.tensor.matmul(
                    r_ps,
                    sp_stack[:, bt * 128 : (bt + 1) * 128],
                    ab_t,
                    start=True,
                    stop=True,
                )
                r_t = rpool.tile([128, E], F32, tag="r", name=f"r_{bt}")
                nc.scalar.activation(r_t, r_ps, AF.Exp, scale=-1.0)
                r_ts[bt] = r_t

        s0_ts = {}
        ble_ts = {}

        def expert(e, x_t, w_t, accs, x8_t, w8_t, bts=None):
            ble_t = None
            if add_bl:
                if e not in ble_ts:
                    ble_t = smallp.tile([1, OUT_F], MDT, tag="ble", bufs=2)
                    nc.sync.dma_start(out=ble_t, in_=bl_d[e : e + 1, :])
                    ble_ts[e] = ble_t
                ble_t = ble_ts[e]
            groups = bts if bts is not None else [
                (bt, bt + 1) for bt in range(0, BT, 2)
            ]
            if e == E - 1 and bts is None:
                # singleton final groups: tile 14's eviction chain overlaps
                # tile 15's matmuls instead of serializing after them
                groups = groups[:-1] + [(BT - 2,), (BT - 1,)]
            for btg in groups:
                hyb = btg[0] < HI_T
                # stationary-reuse order: kc-outer, bt-inner, so each
                # 256-row DR weight load streams two 512-col passes (a DR
                # load costs ~40ns over the 213ns stream when fresh per
                # matmul; amortized over a pair it hides completely)
                lp = {bt: lps.tile([128, OUT_F], F32, tag="lp", name=f"lp_{bt}") for bt in btg}
                bsl = {bt: slice(bt * 128, (bt + 1) * 128) for bt in btg}
                for oc in range(2):
                    osl = slice(oc * 512, (oc + 1) * 512)
                    if hyb:
                        for kc in range(KC8):
                            for bt in btg:
                                nc.tensor.matmul(
                                    lp[bt][:, osl],
                                    x8_t[:, kc, :, bsl[bt]],
                                    w8_t[:, kc, :, osl],
                                    start=(kc == 0),
                                    stop=False,
                                    perf_mode=DR,
                                )
                        for c in range(IC16):
                            for bt in btg:
                                mm(
                                    lp[bt][:, osl],
                                    x_t[:, ICR - IC16 + c, bsl[bt]],
                                    w_t[:, c, osl],
                                    start=False,
                                    stop=(c == IC16 - 1 and not add_bl),
                                )
                    else:
                        for kc in range(KCF):
                            for bt in btg:
                                nc.tensor.matmul(
                                    lp[bt][:, osl],
                                    x8_t[:, kc, :, bsl[bt]],
                                    w8_t[:, kc, :, osl],
                                    start=(kc == 0),
                                    stop=(kc == KCF - 1 and not add_bl),
                                    perf_mode=DR,
                                )
                    if add_bl:
                        for bt in btg:
                            mm(
                                lp[bt][:, osl],
                                ones_t[0:1, 0:128],
                                ble_t[0:1, osl],
                                start=False,
                                stop=True,
                            )
                for bt in btg:
                    target = accs[bt] if e == 0 else None
                    if e != 0:
                        # f16 exp values: halves ScalarE write + DVE read
                        # traffic; quantization (5e-4 rel on ~unit values) is
                        # negligible next to the fp8 noise budget
                        target = expp.tile(
                            [128, OUT_F], F16, tag="exp", name="exp_t"
                        )
                    sh = smallp.tile(
                        [128, 1], F32,
                        tag=("s0h" if e == 0 else "sh"),
                        bufs=(BT + 2 if e == 0 else 6),
                        name=f"sh_{e == 0}",
                    )
                    # ONE fused activation (exp+accum over both 512-col
                    # halves of the two-bank PSUM tile) evicts the block
                    nc.scalar.activation(
                        target, lp[bt], AF.Exp, scale=LOGIT_SCALE, accum_out=sh
                    )
                    if e == 0:
                        # Pure-ScalarE eviction: park unscaled exp in the f16
                        # accumulator; the routing/softmax scale is applied at
                        # e==1, so expert 0 needs no routing result.
                        s0_ts[bt] = sh
                        continue
                    if e == 1:
                        # deferred scale of expert 0's parked contribution
                        d_t = smallp.tile([128, 1], F32, tag="d")
                        nc.vector.reciprocal(d_t, s0_ts[bt])
                        d2_t = smallp.tile([128, 1], F32, tag="d2")
                        nc.vector.tensor_scalar_mul(
                            d2_t, d_t, r_ts[bt][:, 0:1]
                        )
                        nc.vector.tensor_scalar_mul(accs[bt], accs[bt], d2_t)
                    exp_t = target
                    sc_t = smallp.tile([128, 1], F32, tag="sc")
                    nc.vector.reciprocal(sc_t, sh)
                    sc2_t = smallp.tile([128, 1], F32, tag="sc2")
                    nc.vector.tensor_scalar_mul(
                        sc2_t, sc_t, r_ts[bt][:, e : e + 1]
                    )
                    nc.vector.scalar_tensor_tensor(
                        accs[bt], exp_t, sc2_t, accs[bt],
                        op0=ALU.mult, op1=ALU.add,
                    )
                    if e == E - 1:
                        # single full-width writeback (2KB DRAM lines) on the
                        # otherwise-idle gpsimd queue: DIRECT2D descriptor
                        # generation (~600ns/trigger) was stalling the scalar
                        # sequencer in front of the EXPs
                        r0 = bt * 128
                        if bt >= BT - 2:
                            # final tiles: halve the closing DMA by splitting
                            # across the (now idle) sync queue and gpsimd
                            nc.sync.dma_start(
                                out=out_d[r0 : r0 + 128, 0:512],
                                in_=accs[bt][:, 0:512],
                            )
                            nc.gpsimd.dma_start(
                                out=out_d[r0 : r0 + 128, 512:OUT_F],
                                in_=accs[bt][:, 512:OUT_F],
                            )
                        else:
                            nc.gpsimd.dma_start(
                                out=out_d[r0 : r0 + 128, :], in_=accs[bt]
                            )

        def alloc_accs():
            return [
                accp.tile([128, OUT_F], F16, tag="acc", name=f"acc_{bt}")
                for bt in range(BT)
            ]

        # ---- PE warm-up: ~4.5us of throwaway matmuls on a memset tile (no
        # DMA dependency, starts immediately) flips the HAM clock gate to
        # 8/8 before the real Z matmuls start, and fills the PE-idle window
        # while x streams in.
        warm_w = consts.tile([128, 8 * N_NODES], MDT)
        nc.vector.memset(warm_w, 1.0)
        warm_ps = lps.tile([128, OUT_F], F32, tag="lp", name="warm_ps")[0:N_NODES, 0 : 7 * N_NODES]
        NWARM = 24  # run until ~16.5us: the first x16 chunk lands ~17,
        # and a >3.4us PE idle would drop the HAM clock to half speed
        for i in range(NWARM):
            mm(
                warm_ps,
                warm_w[:, 0:N_NODES],
                warm_w[:, N_NODES : 8 * N_NODES],
                start=(i == 0),
                stop=(i == NWARM - 1),
            )
        junk_t = smallp.tile([N_NODES, 1], F32, tag="junk")
        nc.vector.reduce_max(junk_t, warm_ps, axis=mybir.AxisListType.X)

        # ---- emission order tuned so the PE never waits after startup ----
        # The startup is DMA-bound (~8MB must land before expert 1 can
        # stream full-rate), so routing z-chunks and expert-0 blocks
        # interleave, each emitted right after its own DMA pieces: the PE
        # starts on routing bc0 after ~1.1MB and never idles long enough
        # for the HAM clock gate to re-throttle.  Expert 0 needs no
        # routing result (it parks unscaled exp in the accumulator), so it
        # can run before routing completes.
        # queue FIFOs: scalar = [wd, x bc0, x bc2, w16_0];
        # sync = [x bc1, x bc3, x8 kc0-1, w8_0 kc0-1, x8 kc2-3, w8_0 kc2-3,
        #         w8_1, ...]
        x0 = alloc_x()
        x8 = alloc_x8()
        dma_x8_kc(x8, [0])  # 0.25MB, gates the 4 routing DR matmuls
        dma_x(x0)
        dma_x8_kc(x8, [1])
        w800 = dma_w8(0, kcs=range(KC8))
        dma_x8_kc(x8, range(KC8, KCF))
        dma_w8_into(w800, 0, range(KC8, KCF))
        w00 = dma_w(0)  # gpsimd queue
        accs0 = alloc_accs()
        routing_zs(x0, x8)
        expert(0, x0, w00, accs0, x8, w800)
        # routing-r runs AFTER expert 0 (only expert 1+ needs r): off the
        # DMA-saturated startup window, and merged to ONE matmul per batch
        # tile ([sp_neg; sp_pos] stacked on 126 partitions against [A; B])
        routing_r()
        for e in range(1, E):
            expert(e, x0, dma_w(e), accs0, x8, dma_w8(e))

    nc.compile()
    return nc


def _routing_host(x, Wd, bd):
    """Replicate the device routing numerics (fp8 K[0:256] + f16 rest)."""
    import ml_dtypes

    f8 = ml_dtypes.float8_e4m3fn
    x8r = np.clip(x[:, :K8R] * SX, -240, 240).astype(f8).astype(np.float32)
    wd8 = np.clip(Wd[:, :K8R] * SWD, -240, 240).astype(f8).astype(np.float32)
    x16 = x[:, K8R:].astype(np.float16).astype(np.float32)
    wd16 = (Wd[:, K8R:] * (SX * SWD)).astype(np.float16).astype(np.float32)
    z = (x8r @ wd8.T + x16 @ wd16.T) * (1.0 / (SX * SWD))
    if bd is not None and np.any(bd):
        z = z + bd.astype(np.float16).astype(np.float32)
    sp = np.log(np.exp(z) + 1.0)  # softplus(z)
    logr = -((sp - z) @ A_FULL + sp @ B_FULL)
    return np.exp(logr)  # [B, 64]


def make_inputs(x, Wd, bd, Wl, bl, add_bd, add_bl):
    """Per-core input maps + row permutations (adaptive precision classes)."""
    import ml_dtypes

    ndt = np.float16
    f8 = ml_dtypes.float8_e4m3fn

    r = _routing_host(x, Wd, bd)
    # rank rows per core by max leaf routing weight; top HI_T*128 rows get
    # the hybrid (lower-noise) path
    metric = r.reshape(BATCH, N_CORES, E).max(axis=2)  # [B, cores]

    wdT = (
        (Wd[:, K8R:] * (SX * SWD)).T.astype(ndt)
        .reshape(ICR, 128, N_NODES)
        .transpose(1, 0, 2)
    )
    wdT = np.ascontiguousarray(wdT)
    # wd8T[p, i, n] = fp8(SWD * Wd[n, i*128 + p]), zero-padded to 64 nodes
    wd8p = np.zeros((64, K8R), np.float32)
    wd8p[:N_NODES] = np.clip(Wd[:, :K8R] * SWD, -240, 240)
    wd8T = np.ascontiguousarray(
        wd8p.astype(f8).T.reshape(2, 128, 64).transpose(1, 0, 2)
    )
    x8_full = np.clip(x * SX, -240, 240).astype(f8)  # [B, K]

    in_maps, perms = [], []
    for core in range(N_CORES):
        lo, hi = core * E, (core + 1) * E
        perm = np.argsort(-metric[:, core], kind="stable")
        xg = x[perm]
        # SBUF-tile-order layouts: [128, IC, cols] with 1KB contiguous lines
        xT = xg.T[K8R:].astype(ndt).reshape(ICR, 128, BATCH).transpose(1, 0, 2)
        # [K, B]: x8T[k, b] = fp8(SX * xg[b, k]) ; device views rows
        # k = kc*256 + i*128 + p in [128, KCF, 2, B] tile order
        x8T = x8_full[perm].T
        m = {
            "xT": np.ascontiguousarray(xT),
            "wdT": wdT,
            "wd8T": wd8T,
            "x8T": np.ascontiguousarray(x8T),
            "amat": np.ascontiguousarray(A_FULL[:, lo:hi]),
            "bmat": np.ascontiguousarray(B_FULL[:, lo:hi]),
            # f16 half carries 1024*W so both paths land at 1024x logit scale
            "wl": np.ascontiguousarray(
                (Wl[lo:hi, :, K8:] * (SX * SW)).transpose(0, 2, 1).astype(ndt)
            ),
        }
        # [E, KCF, 128, 2, OUT_F]: w8[e,kc,p,i,o] = fp8(SW*W[e,o,kc*256+i*128+p])
        w8 = np.clip(Wl[lo:hi] * SW, -240, 240).astype(f8)
        m["wl8"] = np.ascontiguousarray(
            w8.transpose(0, 2, 1)
            .reshape(E, KCF, 2, 128, OUT_F)
            .transpose(0, 1, 3, 2, 4)
        )
        if add_bd:
            m["bd"] = np.ascontiguousarray(bd.astype(ndt)).reshape(1, N_NODES)
        if add_bl:
            m["bl"] = np.ascontiguousarray((bl[lo:hi] * (SX * SW)).astype(ndt))
        in_maps.append(m)
        perms.append(perm)
    return in_maps, perms


_NC_CACHE = {}


def _get_nc(add_bd, add_bl):
    key = (add_bd, add_bl)
    if key not in _NC_CACHE:
        _NC_CACHE[key] = build_nc(add_bd, add_bl)
    return _NC_CACHE[key]


def run_spmd(x, Wd, bd, Wl, bl, trace=False):
    add_bd = bool(np.any(bd))
    add_bl = bool(np.any(bl))
    nc = _get_nc(add_bd, add_bl)
    in_maps, perms = make_inputs(x, Wd, bd, Wl, bl, add_bd, add_bl)
    res = run_bass_kernel_spmd(nc, in_maps, core_ids=list(range(N_CORES)), trace=trace)
    out = np.zeros((BATCH, OUT_F), np.float64)
    for core, r in enumerate(res.results):
        part = np.empty((BATCH, OUT_F), np.float32)
        part[perms[core]] = r["out"].astype(np.float32)
        out += part
    return out.astype(np.float32), res


def kernel(x, Wd, bd, Wl, bl):
    out, _ = run_spmd(
        np.asarray(x), np.asarray(Wd), np.asarray(bd), np.asarray(Wl), np.asarray(bl)
    )
    return out
